# revision 7
# baseline (speedup 1.0000x reference)
"""Trainium2 Bass kernel for nn_Eq2to2 (Maron et al. equivariant 2->2 layer).

Math (per batch n, with x[d,i,j] = inputs[n,i,j,d], W_b = coefs[:,:,b]):
  out[n,i,j,s] = LeakyReLU( sum_d W9[d,s] x[d,i,j] + W10[d,s] x[d,j,i]
                 + U[j,s] + V[i,s] + G[s] + bias[s]
                 + [i==j] (Dd[i,s] + E[s] + diag_bias[s]) )
  U = c@W5 + r@W6 + diag@W12, V = c@W7 + r@W8 + diag@W11
  Dd = diag@W0 + r@W2 + c@W3, G = tr@W13 + S@W14, E = tr@W1 + S@W4
  (r/c/diag = row sums / col sums / diagonal as [m, d]; tr/S totals)

Sharding (quadrant scheme): 8 cores = batch (4) x quadrant-pair (2).
Each core owns a transpose-CLOSED set of output positions, so both the
W9 path (x[i,j]) and the W10 path (x[j,i]) only touch that core's two
quadrants of x:
  q=0: diagonal quadrants  A = [0:64)x[0:64),   D = [64:128)x[64:128)
  q=1: off-diagonal        B = [0:64)x[64:128), C = [64:128)x[0:64)
All reduced terms (U/V/G/E/Dd) are computed on the host in fp32 (tiny)
and shipped as bf16 [S,128] vectors; the device only runs the two dense
mains + broadcast adds + LeakyReLU.

Device program per core (uniform across cores):
  xin [128, 16 banks x (straight 512 | transposed 512)] fp8e4m3 (2 MiB)
  per bank (8 rows x 64 cols of an output quadrant):
    PE:  psum  = W9 @ straight + W10 @ transposed + ident @ (U bcast)
    DVE: tmp   = psum + (V+G+bias bcast)          (bf16)
    Pool: tmp[diag positions] += (Dd+E+dbias)     (zeros on q=1 cores)
    ACT: out   = LeakyReLU(tmp)                   (bf16)
    DMA out [S, 512]
Host un-permutes the [S, 8192] per-core outputs into [B, M, M, S].
"""

import sys

if "/opt/trn_rl_repo" not in sys.path:
    sys.path.insert(0, "/opt/trn_rl_repo")

import numpy as np
import ml_dtypes

import concourse.bass as bass
import concourse.tile as tile
from concourse import bacc, mybir
from concourse.bass_utils import run_bass_kernel_spmd

B, M, D, S = 4, 128, 128, 128
H = 64               # quadrant side
NBANK = 16           # psum banks of 512 outputs (8 rows x 64 cols)
NCORES = 8
F32 = mybir.dt.float32
BF16 = mybir.dt.float16
F8 = mybir.dt.float8e4
AF = mybir.ActivationFunctionType
NEG_SLOPE = 0.01
BF_NP = np.float16
F8_NP = ml_dtypes.float8_e4m3


def _build_kernel():
    nc = bacc.Bacc(
        "TRN2", target_bir_lowering=False, debug=False, num_devices=NCORES
    )
    xind = nc.dram_tensor("xin", [D, NBANK * 1024], F8, kind="ExternalInput")
    # packed small tensors: [ident | u | vb | dc], one DMA
    smd = nc.dram_tensor("smalls", [M, 4 * M], BF16, kind="ExternalInput")
    # W9/W10 interleaved for the DoubleRow K=256 main: [d, (i, s)]
    wdrd = nc.dram_tensor("wdr", [D, 2 * S], F8, kind="ExternalInput")
    out_t = nc.dram_tensor("out", [S, NBANK, 512], BF16, kind="ExternalOutput")

    with tile.TileContext(nc) as tc:
        _kernel_body(tc, nc, xind, smd, wdrd, out_t)

    nc.compile()
    return nc


def _kernel_body(tc, nc, xind, smd, wdrd, out_t):
    with (
        tc.tile_pool(name="const", bufs=1) as constp,
        tc.tile_pool(name="xt", bufs=1) as xtp,
        tc.tile_pool(name="psum", bufs=7, space="PSUM") as ppool,
        tc.tile_pool(name="warm", bufs=1, space="PSUM") as warmp,
        tc.tile_pool(name="tmp", bufs=6) as tmppool,
        tc.tile_pool(name="osb", bufs=3) as opool,
    ):
        smalls = constp.tile([M, 4 * M], BF16)
        ident = smalls[:, 0:M]
        u_sb = smalls[:, M:2 * M]
        vb_sb = smalls[:, 2 * M:3 * M]
        dc_sb = smalls[:, 3 * M:4 * M]
        wdr = constp.tile([D, 2 * S], F8)
        xin = xtp.tile([D, NBANK * 1024], F8)

        # packed small tensors first (scalar HWDGE ring), then x chunks
        nc.scalar.dma_start(smalls[:], smd.ap())
        nc.scalar.dma_start(wdr[:], wdrd.ap())
        CH = 4 * 1024          # one chunk = 4 banks
        for g in range(4):
            eng = nc.sync if g < 2 else nc.scalar
            eng.dma_start(
                xin[:, g * CH:(g + 1) * CH], xind.ap()[:, g * CH:(g + 1) * CH]
            )

        # PE clock warmup + early Lrelu table load on a memset scratch
        # (no DMA dependency, so these schedule immediately)
        wsc = constp.tile([M, M], BF16)
        nc.vector.memset(wsc[:], 0.0)
        nc.scalar.activation(wsc[:, 8:16], wsc[:, 0:8], AF.Lrelu,
                             alpha=NEG_SLOPE)
        pw = warmp.tile([M, M], BF16)
        for _ in range(12):
            nc.tensor.transpose(pw[:], wsc[:], wsc[:])

        x4 = xin[:].rearrange("d (b i c) -> d b i c", b=NBANK, i=2)
        wdr3 = wdr[:].rearrange("d (i s) -> d i s", i=2)
        DR = mybir.MatmulPerfMode.DoubleRow
        for grp in range(4):
            banks = range(4 * grp, 4 * grp + 4)
            ps = {}
            for b in banks:
                # both mains in one K=256 fp8 DoubleRow matmul
                p = ppool.tile([S, 512], F32)
                nc.tensor.matmul(
                    p[:], wdr3, x4[:, b], start=True, stop=False,
                    perf_mode=DR,
                )
                ps[b] = p
            for b in banks:
                h = b // 8
                ubc = u_sb[:, h * H:(h + 1) * H].unsqueeze(1).broadcast_to(
                    [S, 8, H]
                )
                nc.tensor.matmul(
                    ps[b][:].rearrange("s (r c) -> s r c", r=8),
                    ident, ubc, start=False, stop=True,
                )
            osb = opool.tile([S, 4 * 512], BF16)
            for j, b in enumerate(banks):
                h, k = b // 8, b % 8
                a0 = h * H + 8 * k
                vbc = vb_sb[:, a0:a0 + 8].unsqueeze(2).broadcast_to(
                    [S, 8, H]
                )
                tmp = tmppool.tile([S, 512], BF16)
                nc.vector.tensor_add(
                    tmp[:].rearrange("s (r c) -> s r c", r=8),
                    ps[b][:].rearrange("s (r c) -> s r c", r=8),
                    vbc,
                )
                # diagonal correction: positions r*64 + (8k + r), r<8
                dv = tmp[:, 8 * k:8 * k + 7 * 65 + 1:65]
                nc.gpsimd.tensor_add(dv, dv, dc_sb[:, a0:a0 + 8])
                nc.scalar.activation(osb[:, j * 512:(j + 1) * 512], tmp[:],
                                     AF.Lrelu, alpha=NEG_SLOPE)
            # one 4 KB/partition DMA per group of 4 banks
            nc.sync.dma_start(
                out_t.ap()[:, 4 * grp:4 * grp + 4, :],
                osb[:].rearrange("s (j c) -> s j c", j=4),
            )


_CACHE = {}


def _get_nc():
    if "nc" not in _CACHE:
        _CACHE["nc"] = _build_kernel()
    return _CACHE["nc"]


def _index_sets(q, h):
    if q == 0:
        iset = jset = np.arange(h * H, (h + 1) * H)
    elif h == 0:
        iset, jset = np.arange(0, H), np.arange(H, M)
    else:
        iset, jset = np.arange(H, M), np.arange(0, H)
    return iset, jset


def make_in_maps(inputs, coefs, bias, diag_bias):
    eye = np.eye(M, dtype=np.float32)
    # [d, (i, s)]: W9 at i=0, W10 at i=1 (DoubleRow stationary)
    wdr_np = np.ascontiguousarray(
        np.stack([coefs[:, :, 9], coefs[:, :, 10]], axis=1).reshape(D, 2 * S)
    ).astype(F8_NP)
    W = [coefs[:, :, b] for b in range(15)]

    in_maps = []
    for core in range(NCORES):
        n, q = core // 2, core % 2
        xd = np.ascontiguousarray(inputs[n].transpose(2, 0, 1))  # [d, i, j]
        r_ = xd.sum(axis=2)                 # [d, i]
        c_ = xd.sum(axis=1)                 # [d, j]
        dg = np.einsum('dii->di', xd)       # [d, i]
        tr = dg.sum(axis=1)
        tot = r_.sum(axis=1)
        U = c_.T @ W[5] + r_.T @ W[6] + dg.T @ W[12]    # [j, s]
        V = c_.T @ W[7] + r_.T @ W[8] + dg.T @ W[11]    # [i, s]
        Dd = dg.T @ W[0] + r_.T @ W[2] + c_.T @ W[3]    # [i, s]
        G = tr @ W[13] + tot @ W[14]
        E = tr @ W[1] + tot @ W[4]
        vbf = V + G[None, :] + bias[None, :]
        dcf = Dd + E[None, :] + diag_bias[None, :]

        xdT = xd.transpose(0, 2, 1)
        xin = np.empty((D, NBANK, 2, 512), dtype=F8_NP)
        sm = np.zeros((M, 4 * M), dtype=np.float32)
        sm[:, 0:M] = eye
        for h in range(2):
            iset, jset = _index_sets(q, h)
            st = xd[np.ix_(np.arange(D), iset, jset)]    # [d, 64, 64]
            tp = xdT[np.ix_(np.arange(D), iset, jset)]   # x[d, j, i]
            xin[:, 8 * h:8 * h + 8, 0, :] = st.reshape(D, 8, 512).astype(F8_NP)
            xin[:, 8 * h:8 * h + 8, 1, :] = tp.reshape(D, 8, 512).astype(F8_NP)
            sm[:, M + h * H:M + (h + 1) * H] = U[jset, :].T
            sm[:, 2 * M + h * H:2 * M + (h + 1) * H] = vbf[iset, :].T
            if q == 0:
                sm[:, 3 * M + h * H:3 * M + (h + 1) * H] = dcf[iset, :].T

        in_maps.append({
            "xin": np.ascontiguousarray(xin.reshape(D, NBANK * 1024)),
            "smalls": sm.astype(BF_NP),
            "wdr": wdr_np,
        })
    return in_maps


def kernel(inputs, coefs, bias, diag_bias):
    inputs = np.ascontiguousarray(np.asarray(inputs, dtype=np.float32))
    coefs = np.asarray(coefs, dtype=np.float32)
    bias = np.asarray(bias, dtype=np.float32).reshape(-1)
    diag_bias = np.asarray(diag_bias, dtype=np.float32).reshape(-1)

    nc = _get_nc()
    in_maps = make_in_maps(inputs, coefs, bias, diag_bias)
    # the runtime occasionally reports a transient device-unrecoverable
    # state left over from a previous process; a retry clears it
    last_exc = None
    for attempt in range(3):
        try:
            res = run_bass_kernel_spmd(
                nc, in_maps, core_ids=list(range(NCORES))
            )
            break
        except Exception as e:  # noqa: BLE001
            last_exc = e
            import time as _time
            _time.sleep(10 * (attempt + 1))
    else:
        raise last_exc

    out = np.empty((B, M, M, S), dtype=np.float32)
    for core in range(NCORES):
        n, q = core // 2, core % 2
        r = res.results[core]["out"].astype(np.float32)  # [S, 16, 512]
        r = r.reshape(S, 2, 8, 8, H)
        for h in range(2):
            iset, jset = _index_sets(q, h)
            blk = r[:, h].reshape(S, H, H)               # [s, a, c]
            out[n][np.ix_(iset, jset)] = blk.transpose(1, 2, 0)
    return out


# revision 13
# speedup vs baseline: 1.0529x; 1.0529x over previous
"""Trainium2 Bass kernel for nn_Eq2to2 (Maron et al. equivariant 2->2 layer).

Math (per batch n, with x[d,i,j] = inputs[n,i,j,d], W_b = coefs[:,:,b]):
  out[n,i,j,s] = LeakyReLU( sum_d W9[d,s] x[d,i,j] + W10[d,s] x[d,j,i]
                 + U[j,s] + V[i,s] + G[s] + bias[s]
                 + [i==j] (Dd[i,s] + E[s] + diag_bias[s]) )
  U = c@W5 + r@W6 + diag@W12, V = c@W7 + r@W8 + diag@W11
  Dd = diag@W0 + r@W2 + c@W3, G = tr@W13 + S@W14, E = tr@W1 + S@W4
  (r/c/diag = row sums / col sums / diagonal as [m, d]; tr/S totals)

Sharding (quadrant scheme): 8 cores = batch (4) x quadrant-pair (2).
Each core owns a transpose-CLOSED set of output positions, so both the
W9 path (x[i,j]) and the W10 path (x[j,i]) only touch that core's two
quadrants of x:
  q=0: diagonal quadrants  A = [0:64)x[0:64),   D = [64:128)x[64:128)
  q=1: off-diagonal        B = [0:64)x[64:128), C = [64:128)x[0:64)
All reduced terms (U/V/G/E/Dd) are computed on the host in fp32 (tiny)
and shipped as bf16 [S,128] vectors; the device only runs the two dense
mains + broadcast adds + LeakyReLU.

Device program per core (uniform across cores):
  xin [128, 16 banks x (straight 512 | transposed 512)] fp8e4m3 (2 MiB)
  per bank (8 rows x 64 cols of an output quadrant):
    PE:  psum  = W9 @ straight + W10 @ transposed + ident @ (U bcast)
    DVE: tmp   = psum + (V+G+bias bcast)          (bf16)
    Pool: tmp[diag positions] += (Dd+E+dbias)     (zeros on q=1 cores)
    ACT: out   = LeakyReLU(tmp)                   (bf16)
    DMA out [S, 512]
Host un-permutes the [S, 8192] per-core outputs into [B, M, M, S].
"""

import sys

if "/opt/trn_rl_repo" not in sys.path:
    sys.path.insert(0, "/opt/trn_rl_repo")

import numpy as np
import ml_dtypes

import concourse.bass as bass
import concourse.tile as tile
from concourse import bacc, mybir
from concourse.bass_utils import run_bass_kernel_spmd

B, M, D, S = 4, 128, 128, 128
H = 64               # quadrant side
NBANK = 16           # psum banks of 512 outputs (8 rows x 64 cols)
NCORES = 8
F32 = mybir.dt.float32
BF16 = mybir.dt.float16
F8 = mybir.dt.float8e4
AF = mybir.ActivationFunctionType
NEG_SLOPE = 0.01
BF_NP = np.float16
F8_NP = ml_dtypes.float8_e4m3


def _build_kernel():
    nc = bacc.Bacc(
        "TRN2", target_bir_lowering=False, debug=False, num_devices=NCORES
    )
    xind = nc.dram_tensor("xin", [D, NBANK * 1024], F8, kind="ExternalInput")
    # packed small tensors: [ident | u | vb | dc | wdr(fp8, bitcast)]
    smd = nc.dram_tensor("smalls", [M, 5 * M], BF16, kind="ExternalInput")
    out_t = nc.dram_tensor("out", [S, NBANK, 512], BF16, kind="ExternalOutput")

    with tile.TileContext(nc) as tc:
        _kernel_body(tc, nc, xind, smd, out_t)

    nc.compile()
    return nc


def _kernel_body(tc, nc, xind, smd, out_t):
    with (
        tc.tile_pool(name="const", bufs=1) as constp,
        tc.tile_pool(name="xt", bufs=1) as xtp,
        tc.tile_pool(name="psum", bufs=7, space="PSUM") as ppool,
        tc.tile_pool(name="warm", bufs=1, space="PSUM") as warmp,
        tc.tile_pool(name="tmp", bufs=6) as tmppool,
        tc.tile_pool(name="osb", bufs=3) as opool,
    ):
        smalls = constp.tile([M, 5 * M], BF16)
        ident = smalls[:, 0:M]
        u_sb = smalls[:, M:2 * M]
        vb_sb = smalls[:, 2 * M:3 * M]
        dc_sb = smalls[:, 3 * M:4 * M]
        wdr = smalls[:, 4 * M:5 * M].bitcast(F8)   # [D, 2*S] fp8
        xin = xtp.tile([D, NBANK * 1024], F8)

        # one packed small-tensor DMA (scalar ring); x chunks all on the
        # sync ring so they complete strictly in order (FIFO per ring)
        nc.scalar.dma_start(smalls[:], smd.ap())
        CH = 4 * 1024          # one chunk = 4 banks
        for g in range(4):
            nc.sync.dma_start(
                xin[:, g * CH:(g + 1) * CH], xind.ap()[:, g * CH:(g + 1) * CH]
            )

        # PE clock warmup + early Lrelu table load on a memset scratch
        # (no DMA dependency, so these schedule immediately)
        wsc = constp.tile([M, M], BF16)
        nc.vector.memset(wsc[:], 0.0)
        nc.scalar.activation(wsc[:, 8:16], wsc[:, 0:8], AF.Lrelu,
                             alpha=NEG_SLOPE)
        pw = warmp.tile([M, M], BF16)
        for _ in range(12):
            nc.tensor.transpose(pw[:], wsc[:], wsc[:])

        x4 = xin[:].rearrange("d (b i c) -> d b i c", b=NBANK, i=2)
        wdr3 = wdr.rearrange("d (i s) -> d i s", i=2)
        DR = mybir.MatmulPerfMode.DoubleRow
        for grp in range(4):
            banks = range(4 * grp, 4 * grp + 4)
            ps = {}
            for b in banks:
                # both mains in one K=256 fp8 DoubleRow matmul
                p = ppool.tile([S, 512], F32)
                nc.tensor.matmul(
                    p[:], wdr3, x4[:, b], start=True, stop=False,
                    perf_mode=DR,
                )
                ps[b] = p
            for b in banks:
                h = b // 8
                ubc = u_sb[:, h * H:(h + 1) * H].unsqueeze(1).broadcast_to(
                    [S, 8, H]
                )
                nc.tensor.matmul(
                    ps[b][:].rearrange("s (r c) -> s r c", r=8),
                    ident, ubc, start=False, stop=True,
                )
            osb = opool.tile([S, 4 * 512], BF16)
            for j, b in enumerate(banks):
                h, k = b // 8, b % 8
                a0 = h * H + 8 * k
                vbc = vb_sb[:, a0:a0 + 8].unsqueeze(2).broadcast_to(
                    [S, 8, H]
                )
                tmp = tmppool.tile([S, 512], BF16)
                nc.vector.tensor_add(
                    tmp[:].rearrange("s (r c) -> s r c", r=8),
                    ps[b][:].rearrange("s (r c) -> s r c", r=8),
                    vbc,
                )
                # diagonal correction: positions r*64 + (8k + r), r<8
                dv = tmp[:, 8 * k:8 * k + 7 * 65 + 1:65]
                nc.gpsimd.tensor_add(dv, dv, dc_sb[:, a0:a0 + 8])
                nc.scalar.activation(osb[:, j * 512:(j + 1) * 512], tmp[:],
                                     AF.Lrelu, alpha=NEG_SLOPE)
            # one 4 KB/partition DMA per group of 4 banks (scalar ring,
            # so outputs never queue ahead of input chunks)
            nc.scalar.dma_start(
                out_t.ap()[:, 4 * grp:4 * grp + 4, :],
                osb[:].rearrange("s (j c) -> s j c", j=4),
            )


_CACHE = {}


def _get_nc():
    if "nc" not in _CACHE:
        _CACHE["nc"] = _build_kernel()
    return _CACHE["nc"]


def _index_sets(q, h):
    if q == 0:
        iset = jset = np.arange(h * H, (h + 1) * H)
    elif h == 0:
        iset, jset = np.arange(0, H), np.arange(H, M)
    else:
        iset, jset = np.arange(H, M), np.arange(0, H)
    return iset, jset


def make_in_maps(inputs, coefs, bias, diag_bias):
    eye = np.eye(M, dtype=np.float32)
    # [d, (i, s)]: W9 at i=0, W10 at i=1 (DoubleRow stationary)
    wdr_np = np.ascontiguousarray(
        np.stack([coefs[:, :, 9], coefs[:, :, 10]], axis=1).reshape(D, 2 * S)
    ).astype(F8_NP)
    W = [coefs[:, :, b] for b in range(15)]

    in_maps = []
    for core in range(NCORES):
        n, q = core // 2, core % 2
        xd = np.ascontiguousarray(inputs[n].transpose(2, 0, 1))  # [d, i, j]
        r_ = xd.sum(axis=2)                 # [d, i]
        c_ = xd.sum(axis=1)                 # [d, j]
        dg = np.einsum('dii->di', xd)       # [d, i]
        tr = dg.sum(axis=1)
        tot = r_.sum(axis=1)
        U = c_.T @ W[5] + r_.T @ W[6] + dg.T @ W[12]    # [j, s]
        V = c_.T @ W[7] + r_.T @ W[8] + dg.T @ W[11]    # [i, s]
        Dd = dg.T @ W[0] + r_.T @ W[2] + c_.T @ W[3]    # [i, s]
        G = tr @ W[13] + tot @ W[14]
        E = tr @ W[1] + tot @ W[4]
        vbf = V + G[None, :] + bias[None, :]
        dcf = Dd + E[None, :] + diag_bias[None, :]

        xdT = xd.transpose(0, 2, 1)
        xin = np.empty((D, NBANK, 2, 512), dtype=F8_NP)
        sm = np.zeros((M, 4 * M), dtype=np.float32)
        sm[:, 0:M] = eye
        for h in range(2):
            iset, jset = _index_sets(q, h)
            st = xd[np.ix_(np.arange(D), iset, jset)]    # [d, 64, 64]
            tp = xdT[np.ix_(np.arange(D), iset, jset)]   # x[d, j, i]
            xin[:, 8 * h:8 * h + 8, 0, :] = st.reshape(D, 8, 512).astype(F8_NP)
            xin[:, 8 * h:8 * h + 8, 1, :] = tp.reshape(D, 8, 512).astype(F8_NP)
            sm[:, M + h * H:M + (h + 1) * H] = U[jset, :].T
            sm[:, 2 * M + h * H:2 * M + (h + 1) * H] = vbf[iset, :].T
            if q == 0:
                sm[:, 3 * M + h * H:3 * M + (h + 1) * H] = dcf[iset, :].T

        smb = np.empty((M, 5 * M), dtype=BF_NP)
        smb[:, 0:4 * M] = sm.astype(BF_NP)
        smb[:, 4 * M:5 * M] = wdr_np.view(BF_NP)   # fp8 pair -> one fp16 slot
        in_maps.append({
            "xin": np.ascontiguousarray(xin.reshape(D, NBANK * 1024)),
            "smalls": np.ascontiguousarray(smb),
        })
    return in_maps


def kernel(inputs, coefs, bias, diag_bias):
    inputs = np.ascontiguousarray(np.asarray(inputs, dtype=np.float32))
    coefs = np.asarray(coefs, dtype=np.float32)
    bias = np.asarray(bias, dtype=np.float32).reshape(-1)
    diag_bias = np.asarray(diag_bias, dtype=np.float32).reshape(-1)

    nc = _get_nc()
    in_maps = make_in_maps(inputs, coefs, bias, diag_bias)
    # the runtime occasionally reports a transient device-unrecoverable
    # state left over from a previous process; a retry clears it
    last_exc = None
    for attempt in range(3):
        try:
            res = run_bass_kernel_spmd(
                nc, in_maps, core_ids=list(range(NCORES))
            )
            break
        except Exception as e:  # noqa: BLE001
            last_exc = e
            import time as _time
            _time.sleep(10 * (attempt + 1))
    else:
        raise last_exc

    out = np.empty((B, M, M, S), dtype=np.float32)
    for core in range(NCORES):
        n, q = core // 2, core % 2
        r = res.results[core]["out"].astype(np.float32)  # [S, 16, 512]
        r = r.reshape(S, 2, 8, 8, H)
        for h in range(2):
            iset, jset = _index_sets(q, h)
            blk = r[:, h].reshape(S, H, H)               # [s, a, c]
            out[n][np.ix_(iset, jset)] = blk.transpose(1, 2, 0)
    return out
